# revision 1
# baseline (speedup 1.0000x reference)
"""Trainium2 Bass kernel for Cafe_RNN decode (26-neighbor argmin solidification).

Full inputs -> shard i-axis across 8 NeuronCores (16 planes each + 1-plane
halo) -> per-core slab program (see build()) -> gather to full output.

The per-core program bit-replicates the fp32 reference:
  d_n = ((ex-rx)^2 + (ey-ry)^2) + (ez-rz)^2  per 26-neighborhood shift,
  argmin with first-index-wins ties via strict < updates in shift order,
  plus the pointwise state/field/euler selection logic.

Boundary handling (zero-padded, non-periodic):
 - i: halo planes arrive as zeros at the global edges -> in-loop candidates
   evaluate to exactly d_zero with value (0,0,0).
 - k: zero pad columns in the E0 slab -> same.
 - j: compute engines cannot read partition-shifted operands (AP start
   partition must be 0/32/64/96), so j+-1 shifted slabs Em/Ep are built by
   SBUF->SBUF DMA (partition-unrestricted). Their out-of-range edge row
   keeps a 1e30 sentinel: (1e30-r)^2 overflows to inf -> never wins.
 - every boundary voxel's zero-neighbor candidate is injected at init:
   best = d_zero + pen3, best_c = 0, pen3 = 0 on boundary rows/cols and
   1e30 in the interior.

Engine split (fp32 bit-exactness of ACT Square / GPSIMD add vs DVE
verified on HW): DVE subs/compares/predicated-selects, ACT squares and
copies, GPSIMD adds and memsets.
"""
import json
import sys
from contextlib import ExitStack

if '/opt/trn_rl_repo' not in sys.path:
    sys.path.insert(0, '/opt/trn_rl_repo')

import numpy as np

import concourse.bass as bass
import concourse.tile as tile
from concourse import mybir

F32 = mybir.dt.float32
U8 = mybir.dt.uint8
OP = mybir.AluOpType
AF = mybir.ActivationFunctionType
BIG = 1e30

SHIFTS = [(di, dj, dk)
          for di in (-1, 0, 1) for dj in (-1, 0, 1) for dk in (-1, 0, 1)
          if not (di == 0 and dj == 0 and dk == 0)]


# ---------------------------------------------------------------------------
# This container's walrus build rejects instructions carrying more than one
# semaphore wait ("Too many sync wait commands"). Tile's wait-assignment
# attaches several. Split the extras onto NoOps inserted just before.
_shim_installed = False


def _split_waits(bir_bytes):
    m = json.loads(bir_bytes)
    ctr = 0
    changed = False
    for fn in m.get("functions", []):
        for blk in fn.get("blocks", []):
            insts = blk.get("instructions")
            if not insts:
                continue
            out = []
            for inst in insts:
                si = inst.get("sync_info")
                waits = (si or {}).get("on_wait") or []
                if len(waits) > 1:
                    changed = True
                    for w in waits[:-1]:
                        ctr += 1
                        out.append({
                            "debug": inst.get("debug", 0),
                            "engine": inst["engine"],
                            "ins": [],
                            "name": f"waitsplit_{ctr}_{inst['name']}",
                            "opcode": "NoOp",
                            "outs": [],
                            "sync_info": {"on_update": [], "on_wait": [w]},
                        })
                    si["on_wait"] = [waits[-1]]
                out.append(inst)
            blk["instructions"] = out
    return json.dumps(m).encode() if changed else bir_bytes


def _install_shim():
    global _shim_installed
    if _shim_installed:
        return
    _shim_installed = True
    import concourse.bass2jax as bass2jax
    import concourse.bass_utils as bass_utils
    orig = getattr(bass_utils.compile_bir_kernel, "__wrapped__",
                   bass_utils.compile_bir_kernel)

    def patched(bir_json, tmpdir, neff_name="file.neff"):
        if isinstance(bir_json, str):
            bir_json = bir_json.encode()
        return orig(_split_waits(bir_json), tmpdir, neff_name)

    bass_utils.compile_bir_kernel = patched
    bass2jax.compile_bir_kernel = patched


# ---------------------------------------------------------------------------
def build(NP=16, NJ=128, NK=128, NI=4, repeat=1):
    assert NP % NI == 0
    KW = NK + 2
    nc = bass.Bass("TRN2", target_bir_lowering=False, debug=False, num_devices=8)

    xs = nc.declare_dram_parameter("xs", [NP + 2, NJ, NK * 5], F32, isOutput=False)
    os_ = nc.declare_dram_parameter("os", [NP, NJ, NK * 8], F32, isOutput=False)
    y = nc.declare_dram_parameter("y", [NP, NJ, NK * 5], F32, isOutput=True)

    with tile.TileContext(nc) as tc, ExitStack() as ctx:
        const = ctx.enter_context(tc.tile_pool(name="const", bufs=1))
        persist = ctx.enter_context(tc.tile_pool(name="persist", bufs=1))
        xpool = ctx.enter_context(tc.tile_pool(name="xpool", bufs=1))
        och = ctx.enter_context(tc.tile_pool(name="och", bufs=2))
        ych = ctx.enter_context(tc.tile_pool(name="ych", bufs=1))
        bestp = ctx.enter_context(tc.tile_pool(name="best", bufs=2))
        temps = ctx.enter_context(tc.tile_pool(name="temps", bufs=2))

        c_neg1 = const.tile([NJ, 1], F32)
        nc.vector.memset(c_neg1[:, :], -1.0)
        c_one = const.tile([NJ, 1], F32)
        nc.vector.memset(c_one[:, :], 1.0)
        c_two = const.tile([NJ, 1], F32)
        nc.vector.memset(c_two[:, :], 2.0)

        pen3 = const.tile([NJ, NI, NK], F32)
        nc.vector.memset(pen3[:], BIG)
        nc.vector.memset(pen3[0:1, :, :], 0.0)
        nc.sync.dma_start(pen3[NJ - 1:NJ, :, :], pen3[0:1, :, :])
        nc.vector.memset(pen3[:, :, 0:1], 0.0)
        nc.vector.memset(pen3[:, :, NK - 1:NK], 0.0)

        E0 = persist.tile([NJ, 3, NP + 2, KW], F32)
        Em = persist.tile([NJ, 3, NP + 2, KW], F32)
        Ep = persist.tile([NJ, 3, NP + 2, KW], F32)
        nc.gpsimd.memset(E0[:, :, :, :], 0.0)
        nc.gpsimd.memset(Em[:, :, :, :], BIG)
        nc.gpsimd.memset(Ep[:, :, :, :], BIG)
        st0s = persist.tile([NJ, NP, NK], F32)

        for p in range(NP + 2):
            Xp = xpool.tile([NJ, NK * 5], F32, tag="xp")
            nc.sync.dma_start(Xp[:, :], xs[p, :, :])
            src = Xp[:, :].rearrange("j (k c) -> j c k", c=5)
            nc.scalar.copy(E0[:, :, p, 1:NK + 1], src[:, 1:4, :])
            if 1 <= p <= NP:
                nc.scalar.copy(st0s[:, p - 1, :], src[:, 0, :])
            nc.sync.dma_start(Em[1:NJ, :, p, 1:NK + 1], E0[0:NJ - 1, :, p, 1:NK + 1])
            nc.sync.dma_start(Ep[0:NJ - 1, :, p, 1:NK + 1], E0[1:NJ, :, p, 1:NK + 1])

        EJ = {-1: Em, 0: E0, 1: Ep}

        def EVall(c0, di, dj, dk):
            # [NJ, 3, NI, NK] all-channel neighbor view
            return EJ[dj][:, :, 1 + c0 + di:1 + c0 + di + NI, 1 + dk:1 + dk + NK]

        sh = [NJ, NI, NK]
        sh3 = [NJ, 3, NI, NK]
        for _rep in range(repeat):
          for c0 in range(0, NP, NI):
            O = och.tile([NJ, NI, NK * 8], F32)
            nc.sync.dma_start(
                O[:, :, :], os_[c0:c0 + NI, :, :].rearrange("i j k -> j i k"))
            Ov = O[:, :, :].rearrange("j i (k c) -> j c i k", c=8)
            l0, l1, l2, l3 = (Ov[:, q] for q in range(4))
            r_all = Ov[:, 4:7]
            rx, ry, rz, f = (Ov[:, q] for q in range(4, 8))
            st0 = st0s[:, c0:c0 + NI, :]

            Y = ych.tile([NJ, NI, NK * 5], F32)
            Yv = Y[:, :, :].rearrange("j i (k c) -> j c i k", c=5)
            y0, y4 = Yv[:, 0], Yv[:, 4]
            y123 = Yv[:, 1:4]

            ta = temps.tile(sh3, F32, tag="ta")
            rc = temps.tile(sh3, F32, tag="rc")
            t1 = temps.tile(sh, F32, tag="t1")
            t3 = temps.tile(sh, F32, tag="t3")
            s1 = temps.tile(sh, F32, tag="s1")
            s2 = temps.tile(sh, F32, tag="s2")
            dd = temps.tile(sh, F32, tag="dd")
            bd = bestp.tile(sh, F32, tag="bd")
            bca = bestp.tile(sh3, F32, tag="bca")

            nc.scalar.copy(rc[:], r_all)
            nc.scalar.square(ta[:], rc[:])
            nc.gpsimd.tensor_tensor(dd[:], ta[:, 0], ta[:, 1], op=OP.add)
            nc.gpsimd.tensor_tensor(dd[:], dd[:], ta[:, 2], op=OP.add)
            nc.gpsimd.tensor_tensor(bd[:], dd[:], pen3[:], op=OP.add)
            nc.gpsimd.memset(bca[:], 0.0)

            for di, dj, dk in SHIFTS:
                e_all = EVall(c0, di, dj, dk)
                mk = temps.tile([NJ, 1, NI, NK], U8, tag="mk")
                nc.vector.tensor_tensor(ta[:], e_all, rc[:], op=OP.subtract)
                nc.scalar.square(ta[:], ta[:])
                nc.gpsimd.tensor_tensor(dd[:], ta[:, 0], ta[:, 1], op=OP.add)
                nc.gpsimd.tensor_tensor(dd[:], dd[:], ta[:, 2], op=OP.add)
                nc.vector.tensor_tensor(mk[:, 0], dd[:], bd[:], op=OP.is_lt)
                nc.vector.copy_predicated(bd[:], mk[:, 0], dd[:])
                nc.vector.copy_predicated(
                    bca[:], mk[:, :, :, :].broadcast_to(sh3), e_all)

            # pointwise epilogue
            nc.vector.tensor_tensor(y0, l1, l0, op=OP.is_gt)
            nc.vector.tensor_tensor(t1[:], l0, l1, op=OP.max)
            nc.vector.tensor_tensor(s2[:], l3, l2, op=OP.is_gt)
            nc.vector.tensor_tensor(t3[:], l2, l3, op=OP.max)
            nc.scalar.activation(s1[:], s2[:], AF.Identity, bias=c_two[:, :], scale=1.0)
            u_sw = temps.tile(sh, U8, tag="u_sw")
            nc.vector.tensor_tensor(u_sw[:], t3[:], t1[:], op=OP.is_gt)
            nc.vector.copy_predicated(y0, u_sw[:], s1[:])
            nc.vector.tensor_scalar(t3[:], st0, 0.5, None, op0=OP.is_ge)
            nc.vector.tensor_tensor(y0, y0, t3[:], op=OP.mult)
            u_le1 = temps.tile([NJ, 1, NI, NK], U8, tag="u_le1")
            nc.vector.tensor_scalar(u_le1[:, 0], y0, 1.5, None, op0=OP.is_le)
            u_eq2 = temps.tile(sh, U8, tag="u_eq2")
            nc.gpsimd.tensor_scalar(u_eq2[:], y0, 2.0, None, op0=OP.is_equal)
            u_ge3 = temps.tile(sh, U8, tag="u_ge3")
            nc.gpsimd.tensor_scalar(u_ge3[:], y0, 2.5, None, op0=OP.is_ge)
            nc.vector.tensor_scalar(t3[:], st0, 1.5, None, op0=OP.is_le)
            nc.vector.tensor_scalar(s1[:], y0, 1.5, None, op0=OP.is_ge)
            u_fs = temps.tile([NJ, 1, NI, NK], U8, tag="u_fs")
            nc.vector.tensor_tensor(u_fs[:, 0], s1[:], t3[:], op=OP.mult)
            nc.scalar.copy(y4, f)
            nc.vector.tensor_scalar(s2[:], f, 0.0, 0.92, op0=OP.max, op1=OP.min)
            nc.vector.copy_predicated(y4, u_eq2[:], s2[:])
            nc.vector.copy_predicated(y4, u_le1[:, 0], c_neg1[:, :].broadcast_to(sh))
            nc.vector.copy_predicated(y4, u_ge3[:], c_one[:, :].broadcast_to(sh))
            nc.scalar.copy(y123, E0[:, :, 1 + c0:1 + c0 + NI, 1:1 + NK])
            nc.vector.copy_predicated(
                y123, u_le1[:].broadcast_to(sh3), c_neg1[:, :].broadcast_to(sh3))
            nc.vector.copy_predicated(y123, u_fs[:].broadcast_to(sh3), bca[:])

            nc.sync.dma_start(
                y[c0:c0 + NI, :, :].rearrange("i j k -> j i k"), Y[:, :, :])
    return nc


_NC = None


def _get_nc():
    global _NC
    if _NC is None:
        _install_shim()
        _NC = build(NP=16, NJ=128, NK=128, NI=4)
    return _NC


def _run(x, out, trace=False):
    from concourse.bass_utils import run_bass_kernel_spmd
    D, NP, NC_ = 128, 16, 8
    x = np.ascontiguousarray(np.asarray(x, dtype=np.float32))
    out = np.ascontiguousarray(np.asarray(out, dtype=np.float32))
    xg = x.reshape(D, D, D * 5)
    og = out.reshape(D, D, D * 8)
    in_maps = []
    for c in range(NC_):
        xs = np.zeros((NP + 2, D, D * 5), np.float32)
        lo = c * NP - 1
        glo, ghi = max(lo, 0), min(c * NP + NP + 1, D)
        xs[glo - lo:ghi - lo] = xg[glo:ghi]
        in_maps.append({"xs": xs,
                        "os": np.ascontiguousarray(og[c * NP:(c + 1) * NP])})
    res = run_bass_kernel_spmd(_get_nc(), in_maps,
                               core_ids=list(range(NC_)), trace=trace)
    yfull = np.concatenate([res.results[c]["y"] for c in range(NC_)], axis=0)
    return yfull.reshape(1, D, D, D, 5), res


def kernel(x, out):
    return _run(x, out)[0]

